# revision 4
# baseline (speedup 1.0000x reference)
"""GCMC message-passing kernel for trn2: builder + host preprocessing.

Per core = one dst-shard, both directions (0: drug->dis, 1: dis->drug).
  Phase W: device computes W[r] = att @ basis -> Wtab[R, IN, 128pad] bf16 HBM.
  Phase E (x6 passes = 2 dirs x 3 k-feats): per-edge event streams sorted by
    slot (r-major, dst-local), 128-event windows, WPP windows per 128-slot
    page, per-rating window count padded to RW (mult of 8) so each 1024-event
    gather call is single-rating. dma_gather pulls 256B W rows from wtab[r]
    (1024 events/call); DVE builds SegT[128ev,128slot] = is_equal(IC,sl)*sc
    (sc = cj*ci, host-folded); PE: msgs.T @ SegT accumulated into a PSUM page
    [MU, 128]. Pages -> SBUF stage (ACT) -> hT[d,k] = [MU, NSLOT] bf16 HBM.
  Phase P: outT[d] [256, SH] = sum_rk fcblk_rk.T @ hT-slices + bias.
Host assembles + transposes the two outputs.

Wire-size optimizations (axon transfer is the wall-clock bottleneck):
  - gather indices: feat only (10 bits), 8 values packed in 5 int16,
    shipped untiled [16, .]; device unpacks with DVE int ops and rebuilds
    the DGE 16-partition-wrapped replicated layout via 8 partition-group
    DMAs. Rating is static per call -> per-rating gather table slice.
  - sl/sc shipped once per direction (identical across the 3 k-passes);
    sl as uint8, sc as bf16; converted to f32 once on device.
  - bf16 everywhere off-chip except f32 PSUM accumulation: basis, fc
    weights, h, outputs.
"""
import numpy as np
import jax
jax.config.update("jax_compilation_cache_dir", "/tmp/jaxcache")
jax.config.update("jax_persistent_cache_min_entry_size_bytes", -1)
jax.config.update("jax_persistent_cache_min_compile_time_secs", 0)
import concourse.bass as bass
import concourse.bacc as bacc
import concourse.mybir as mybir

F32 = mybir.dt.float32
BF16 = mybir.dt.bfloat16
I16 = mybir.dt.int16
U8 = mybir.dt.uint8
NPBF16 = mybir.dt.np(mybir.dt.bfloat16)

R = 5
MU = 64
OUT = 256
NK = 3


class Cfg:
    def __init__(self, n_nodes, in_units, n_cores, wpp):
        self.N = n_nodes
        self.IN = in_units
        self.NC = n_cores
        self.SH = n_nodes // n_cores
        self.PPR = (self.SH + 127) // 128
        self.NPAGE = R * self.PPR
        self.NSLOT = self.NPAGE * 128
        self.WPP = wpp
        self.RW = ((self.PPR * wpp + 7) // 8) * 8   # windows per rating
        self.NWP = R * self.RW
        self.NCALL = self.NWP // 8
        self.CPC = 8                          # gather calls per input chunk
        self.WPC = self.CPC * 8               # windows per chunk
        self.NCHUNK = (self.NCALL + self.CPC - 1) // self.CPC
        self.DCH = 512
        self.NDC = (self.SH + self.DCH - 1) // self.DCH
        self.SPS = 16
        self.NSTG = (self.NPAGE + self.SPS - 1) // self.SPS
        self.WROUND = (in_units * MU) // 2048
        assert (in_units * MU) % 2048 == 0


def _sorted_dirs(inputs):
    """Per direction, per rating: dst-sorted (dst, src)."""
    gi = lambda n: np.asarray(inputs[n], np.int64)
    src, dst = gi("src"), gi("dst")
    out = []
    for dkey, skey in ((dst, src), (src, dst)):
        parts = []
        for r in range(R):
            order = np.argsort(dkey[r], kind="stable")
            parts.append((dkey[r][order], skey[r][order]))
        out.append(parts)
    return out


def max_page_count(sh, ncores, ppr, dirs_sorted):
    """Largest event count landing on one (core, rating, local-page) bucket."""
    mx = 0
    for parts in dirs_sorted:
        for r in range(R):
            dk = parts[r][0]
            core = dk // sh
            key = core * ppr + ((dk - core * sh) >> 7)
            cnt = np.bincount(key, minlength=ncores * ppr)
            mx = max(mx, int(cnt.max()))
    return mx


def build_inputs(cfg, inputs, dirs_sorted):
    f32 = np.float32
    gf = lambda n: np.asarray(inputs[n], f32)
    gi = lambda n: np.asarray(inputs[n], np.int64)
    drug_feat, dis_feat = gi("drug_feat"), gi("dis_feat")
    cj_drug, ci_drug = gf("cj_drug"), gf("ci_drug")
    cj_dis, ci_dis = gf("cj_dis"), gf("ci_dis")
    att, basis = gf("att"), gf("basis")
    fc_w, fc_b = gf("fc_w"), gf("fc_b")
    IN, SH, PPR, WPP, RW = cfg.IN, cfg.SH, cfg.PPR, cfg.WPP, cfg.RW
    NWP, NCALL, NPAGE, NC = cfg.NWP, cfg.NCALL, cfg.NPAGE, cfg.NC

    attT = att.T.astype(NPBF16)
    basisf = basis.reshape(4, IN * MU).astype(NPBF16)
    # fcrT[m, rk, o] = fc_w[r*NK*MU + k*MU + m, o]
    fcrT = fc_w.reshape(R * NK, MU, OUT).transpose(1, 0, 2).astype(NPBF16)
    IC = np.tile(np.arange(128, dtype=f32)[None, :], (128, 1)).copy()

    dirspec = [(drug_feat, cj_drug, ci_dis), (dis_feat, cj_dis, ci_drug)]
    maps = [
        {"attT": attT, "basisf": basisf, "fcrT": fcrT, "ic": IC,
         "g": np.zeros((16, 2 * NK * NCALL * 40), np.int16),
         "sl": np.zeros((128, 2 * NWP), np.uint8),
         "sc": np.zeros((128, 2 * NWP), NPBF16)}
        for _ in range(NC)
    ]
    for d, (feat, cj, ci) in enumerate(dirspec):
        parts = dirs_sorted[d]
        bounds = [np.searchsorted(parts[r][0], np.arange(NC + 1) * SH)
                  for r in range(R)]
        for c in range(NC):
            lo = c * SH
            dcat, scat, rcat = [], [], []
            for r in range(R):
                b0, b1 = bounds[r][c], bounds[r][c + 1]
                dcat.append(parts[r][0][b0:b1])
                scat.append(parts[r][1][b0:b1])
                rcat.append(np.full(b1 - b0, r, np.int64))
            dcat = np.concatenate(dcat)
            scat = np.concatenate(scat)
            rcat = np.concatenate(rcat)
            slot = rcat * (PPR * 128) + (dcat - lo)
            sc = cj[scat, 0] * ci[dcat, 0]
            pageg = slot >> 7                       # rcat*PPR + local page
            counts = np.bincount(pageg, minlength=NPAGE)
            assert counts.max() <= WPP * 128, (
                f"page overflow {counts.max()} > {WPP*128}; raise WPP")
            starts = np.concatenate(([0], np.cumsum(counts)[:-1]))
            pos = np.arange(slot.size) - np.repeat(starts, counts)
            ploc = pageg - rcat * PPR
            w = rcat * RW + ploc * WPP + (pos >> 7)
            flat = w * 128 + (pos & 127)
            SL = np.zeros(NWP * 128, np.uint8)
            SL[flat] = (slot & 127).astype(np.uint8)
            SC = np.zeros(NWP * 128, NPBF16)
            SC[flat] = sc.astype(NPBF16)
            m = maps[c]
            m["sl"][:, d * NWP:(d + 1) * NWP] = SL.reshape(NWP, 128).T
            m["sc"][:, d * NWP:(d + 1) * NWP] = SC.reshape(NWP, 128).T
            for k in range(NK):
                G = np.zeros(NWP * 128, np.uint16)
                G[flat] = feat[scat, k].astype(np.uint16)
                pi = d * NK + k
                wr = G.reshape(NCALL, 64, 16).transpose(2, 0, 1)
                v8 = wr.reshape(16, NCALL, 8, 8)
                vv = [v8[..., i] for i in range(8)]
                s5 = np.empty((16, NCALL, 8, 5), np.uint16)
                s5[..., 0] = vv[0] | (vv[1] << 10)
                s5[..., 1] = (vv[1] >> 6) | (vv[2] << 4) | (vv[3] << 14)
                s5[..., 2] = (vv[3] >> 2) | (vv[4] << 8)
                s5[..., 3] = (vv[4] >> 8) | (vv[5] << 2) | (vv[6] << 12)
                s5[..., 4] = (vv[6] >> 4) | (vv[7] << 6)
                m["g"][:, pi * NCALL * 40:(pi + 1) * NCALL * 40] = (
                    s5.reshape(16, NCALL * 40).view(np.int16))
    return maps


def assemble_output(cfg, results, fc_b):
    outs = []
    for d in range(2):
        parts = []
        for c in range(cfg.NC):
            i8 = results[c]["outT"][d].astype(np.float32)      # [OUT, SH]
            s = np.asarray(results[c]["oscale"][d], np.float32)  # [128, 2]
            scale = np.concatenate([s[:, 0], s[:, 1]]) / 127.0   # [OUT]
            parts.append((i8 * scale[:, None]).T)
        outs.append(np.concatenate(parts, 0) + np.asarray(fc_b, np.float32)[None, :])
    return outs[1], outs[0]


def build_kernel(cfg):
    nc = bacc.Bacc(None, target_bir_lowering=False, debug=True)
    IN, NCALL, NWP, RW = cfg.IN, cfg.NCALL, cfg.NWP, cfg.RW
    NPAGE, WPP, SH, PPR = cfg.NPAGE, cfg.WPP, cfg.SH, cfg.PPR
    CPC, WPC, NCHUNK = cfg.CPC, cfg.WPC, cfg.NCHUNK
    NSLOT, DCH, NDC, SPS, NSTG = cfg.NSLOT, cfg.DCH, cfg.NDC, cfg.SPS, cfg.NSTG
    WROUND = cfg.WROUND
    PASSES = [(d, k) for d in range(2) for k in range(NK)]

    attT_d = nc.declare_dram_parameter("attT", [4, R], BF16, isOutput=False)
    basisf_d = nc.declare_dram_parameter("basisf", [4, IN * MU], BF16, isOutput=False)
    fcr_d = nc.declare_dram_parameter("fcrT", [MU, R * NK, OUT], BF16, isOutput=False)
    ic_d = nc.declare_dram_parameter("ic", [128, 128], F32, isOutput=False)
    g_d = nc.declare_dram_parameter("g", [16, 2 * NK * NCALL * 40], I16, isOutput=False)
    sl_d = nc.declare_dram_parameter("sl", [128, 2 * NWP], U8, isOutput=False)
    sc_d = nc.declare_dram_parameter("sc", [128, 2 * NWP], BF16, isOutput=False)
    outT_d = nc.declare_dram_parameter("outT", [2, OUT, SH], mybir.dt.int8, isOutput=True)
    oscale_d = nc.declare_dram_parameter("oscale", [2, 128, 2], F32, isOutput=True)

    # gather rows must be 256B multiples: pad W rows to 128 bf16 (64 real)
    wtab = nc.dram_tensor("wtab", [R, IN, 128], BF16)
    wtab_r = [wtab[r:r+1].rearrange("r f m -> (r f) m") for r in range(R)]
    hT = nc.dram_tensor("hT", [2, NK, MU, NSLOT], BF16)

    attT_sb = nc.alloc_sbuf_tensor("attT_sb", [4, R], BF16)
    bchunk = nc.alloc_sbuf_tensor("bchunk", [4, 2048], BF16)
    wstage = nc.alloc_sbuf_tensor("wstage", [R, 2048], BF16)
    ic_sb = nc.alloc_sbuf_tensor("ic_sb", [128, 128], F32)
    fcr_sb = nc.alloc_sbuf_tensor("fcr_sb", [MU, R * NK, OUT], BF16)
    NIB = 2
    gbuf = nc.alloc_sbuf_tensor("gbuf", [128, NIB, CPC * 64], I16)
    pbuf = nc.alloc_sbuf_tensor("pbuf", [128, NIB, CPC * 40], I16)
    utmpA = nc.alloc_sbuf_tensor("utmpA", [128, 4, CPC * 8], I16)
    utmpB = nc.alloc_sbuf_tensor("utmpB", [128, 4, CPC * 8], I16)
    slsb = nc.alloc_sbuf_tensor("slsb", [128, 2 * NWP], U8)
    scsb = nc.alloc_sbuf_tensor("scsb", [128, 2 * NWP], BF16)
    slf = nc.alloc_sbuf_tensor("slf", [128, 2 * NWP], F32)
    scf = nc.alloc_sbuf_tensor("scf", [128, 2 * NWP], F32)
    NMB = 4
    msgs = [nc.alloc_sbuf_tensor(f"msgs{i}", [128, 8, 128], BF16) for i in range(NMB)]
    NSB = 4
    segt = [nc.alloc_sbuf_tensor(f"segt{i}", [128, 128], BF16) for i in range(NSB)]
    NSTB = 2
    stage = [nc.alloc_sbuf_tensor(f"stage{i}", [MU, SPS * 128], BF16) for i in range(NSTB)]
    prhs = [nc.alloc_sbuf_tensor(f"prhs{i}", [MU, R * NK, DCH], BF16) for i in range(2)]
    obig = nc.alloc_sbuf_tensor("obig", [128, 2, SH], BF16)
    oq = nc.alloc_sbuf_tensor("oq", [128, 2 * SH], mybir.dt.int8)
    amaxc = nc.alloc_sbuf_tensor("amaxc", [128, 4], F32)
    rscl = nc.alloc_sbuf_tensor("rscl", [128, 4], F32)

    NPB = 4
    psA = nc.alloc_psum_tensor("psA", [128, 2048], F32)
    pages = [psA[0:MU, i * 512:i * 512 + 128] for i in range(NPB)]
    psB = nc.alloc_psum_tensor("psB", [128, 2048], F32)
    wps = psB[0:R, :]
    pps = [psB[:, j * 512:(j + 1) * 512] for j in range(4)]  # j = 2*(l%2)+h

    def page_of_window(w):
        rr = w // RW
        return rr * PPR + min((w % RW) // WPP, PPR - 1)

    def rating_of_call(c):
        return min((c * 8) // RW, R - 1)

    wlast = {}
    for pi in range(len(PASSES)):
        for w in range(NWP):
            wlast[pi, page_of_window(w)] = pi * NWP + w

    def chunk_cols(ch):
        c0 = ch * CPC
        return c0, min(CPC, NCALL - c0)

    # global chunk -> (global calls through chunk, global windows through chunk)
    chk_calls, chk_wins = [], []
    tc = tw = 0
    for pi in range(len(PASSES)):
        for ch in range(NCHUNK):
            c0, ncc = chunk_cols(ch)
            tc += ncc
            tw += ncc * 8
            chk_calls.append(tc)
            chk_wins.append(tw)

    sems = {}

    with nc.Block() as block:
        for name, n in [("gth", NMB), ("inb", NIB), ("stg", NSTB),
                        ("pin", 2)]:
            for i in range(n):
                sems[name, i] = nc.alloc_semaphore(f"s_{name}{i}")
        for name in ["wdma", "wout", "wmm", "wcp", "seg", "pe", "act", "pmm",
                     "oact", "slsc", "upk", "utm", "qv", "qt", "od"]:
            sems[name] = nc.alloc_semaphore(f"s_{name}")
        s_gth = [sems["gth", i] for i in range(NMB)]
        s_inb = [sems["inb", i] for i in range(NIB)]
        s_stg = [sems["stg", i] for i in range(NSTB)]
        s_pin = [sems["pin", i] for i in range(2)]
        s_wdma, s_wmm, s_wcp = sems["wdma"], sems["wmm"], sems["wcp"]
        s_wout = sems["wout"]
        s_seg, s_pe, s_act = sems["seg"], sems["pe"], sems["act"]
        s_pmm, s_oact = sems["pmm"], sems["oact"]
        s_slsc, s_upk, s_utm = sems["slsc"], sems["upk"], sems["utm"]
        s_qv, s_qt, s_od = sems["qv"], sems["qt"], sems["od"]

        # ============ GPSIMD: const + W-build DMAs, then gathers
        @block.gpsimd
        def _(g):
            g.dma_start(attT_sb[:], attT_d[:]).then_inc(s_wdma, 16)
            g.dma_start(ic_sb[:], ic_d[:]).then_inc(s_wdma, 16)
            g.dma_start(fcr_sb[:], fcr_d[:]).then_inc(s_wdma, 16)
            for n in range(WROUND):
                g.wait_ge(s_wcp, n)  # round n-1 psum copied
                g.dma_start(bchunk[:], basisf_d[:, n*2048:(n+1)*2048]).then_inc(s_wdma, 16)
                g.wait_ge(s_wcp, n + 1)
                g.dma_start(wtab[:, n*32:(n+1)*32, 0:MU],
                            wstage[:].rearrange("r (f m) -> r f m", m=MU)
                            ).then_inc(s_wout, 16)
            g.wait_ge(s_wout, WROUND * 16)  # all wtab writes landed
            ncall = 0
            for pi, (d, k) in enumerate(PASSES):
                for c in range(NCALL):
                    gch = pi * NCHUNK + c // CPC
                    g.wait_ge(s_upk, 8 * (gch + 1))
                    b = ncall % NMB
                    if ncall >= NMB:
                        g.wait_ge(s_pe, 8 * (ncall - NMB + 1))
                    g.dma_gather(
                        msgs[b][:], wtab_r[rating_of_call(c)],
                        gbuf[:, gch % NIB, (c % CPC) * 64:(c % CPC + 1) * 64],
                        1024, 1024, 128,
                    ).then_inc(s_gth[b], 16)
                    ncall += 1

        # ============ TENSOR: W MMs, window MMs, projection MMs
        @block.tensor
        def _(t):
            for n in range(WROUND):
                t.wait_ge(s_wdma, 48 + 16 * (n + 1))
                if n >= 1:
                    t.wait_ge(s_wcp, n)
                for i in range(4):
                    ins = t.matmul(wps[:, i*512:(i+1)*512], attT_sb[:],
                                   bchunk[:, i*512:(i+1)*512],
                                   start=True, stop=True)
                ins.then_inc(s_wmm, 1)
            wglob = 0
            for pi, (d, k) in enumerate(PASSES):
                for w in range(NWP):
                    p = page_of_window(w)
                    pglob = pi * NPAGE + p
                    first = ((w % RW) % WPP == 0) and ((w % RW) // WPP == p % PPR)
                    if first and pglob >= NPB:
                        t.wait_ge(s_act, pglob - NPB + 1)
                    b = (wglob // 8) % NMB
                    t.wait_ge(s_gth[b], 16 * (wglob // 8 // NMB + 1))
                    t.wait_ge(s_seg, wglob + 1)
                    t.matmul(pages[pglob % NPB],
                             msgs[b][:, w % 8, 0:MU],
                             segt[wglob % NSB][:],
                             start=first, stop=(wglob == wlast[pi, p]),
                             ).then_inc(s_pe, 1)
                    wglob += 1
            nl = 0
            for d in range(2):
                for c in range(NDC):
                    ncols = min(DCH, SH - c * DCH)
                    t.wait_ge(s_pin[nl % 2], 240 * (nl // 2 + 1))
                    if nl >= 2:
                        t.wait_ge(s_oact, 2 * (nl - 1))
                    for h in range(2):
                        for rk in range(R * NK):
                            ins = t.matmul(pps[2*(nl % 2)+h][:, :ncols],
                                           fcr_sb[:, rk, h*128:(h+1)*128],
                                           prhs[nl % 2][:, rk, :ncols],
                                           start=(rk == 0), stop=(rk == R*NK-1))
                        ins.then_inc(s_pmm, 1)
                    nl += 1

        # ============ VECTOR: W psum->sbuf copies, unpack, SegT builds
        @block.vector
        def _(v):
            for n in range(WROUND):
                v.wait_ge(s_wmm, n + 1)
                if n >= 1:
                    v.wait_ge(s_wout, 16 * n)
                v.tensor_copy(wstage[:], wps[:]).then_inc(s_wcp, 1)
            v.wait_ge(s_slsc, 32)
            v.tensor_copy(slf[:], slsb[:]).then_inc(s_slsc, 1)
            v.tensor_copy(scf[:], scsb[:]).then_inc(s_slsc, 1)
            v.wait_ge(s_slsc, 34)

            band = mybir.AluOpType.bitwise_and
            bor = mybir.AluOpType.bitwise_or
            lsr = mybir.AluOpType.logical_shift_right
            lsl = mybir.AluOpType.logical_shift_left

            def unpack_chunk(gch, ib, ncc):
                v.wait_ge(s_inb[ib], 128 * (gch // NIB + 1))
                if gch >= NIB:
                    v.wait_ge(s_pe, chk_wins[gch - NIB])
                if gch >= 1:
                    v.wait_ge(s_upk, 8 * gch)     # prior chunk writers done
                P = pbuf[:, ib, :ncc*40].rearrange("p (cj f) -> p cj f", f=5)
                Gv = gbuf[:, ib, :ncc*64].rearrange("p (cj q) -> p cj q", q=8)
                CJ = ncc * 8
                # single-source lanes
                v.tensor_scalar(Gv[:, :, 0], P[:, :, 0], 1023, None,
                                band).then_inc(s_upk, 1)
                v.tensor_scalar(Gv[:, :, 2], P[:, :, 1], 4, 1023,
                                lsr, band).then_inc(s_upk, 1)
                v.tensor_scalar(Gv[:, :, 5], P[:, :, 3], 2, 1023,
                                lsr, band).then_inc(s_upk, 1)
                v.tensor_scalar(Gv[:, :, 7], P[:, :, 4], 6, 1023,
                                lsr, band).then_inc(s_upk, 1)
                # two-source lanes: B = (Pb << sb) & 1023; G = (Pa >> sa) | B
                two = [(1, 0, 10, 1, 6), (3, 1, 14, 2, 2),
                       (4, 2, 8, 3, 8), (6, 3, 12, 4, 4)]
                for i, (q, pa, sa, pb, sb) in enumerate(two):
                    v.tensor_scalar(utmpB[:, i, :CJ], P[:, :, pb], sb, 1023,
                                    lsl, band).then_inc(s_utm, 1)
                # mask A to its valid width: arithmetic-vs-logical srl safe
                for i, (q, pa, sa, pb, sb) in enumerate(two):
                    v.tensor_scalar(utmpA[:, i, :CJ], P[:, :, pa], sa,
                                    (1 << (16 - sa)) - 1, lsr,
                                    band).then_inc(s_utm, 1)
                v.wait_ge(s_utm, 8 * (gch + 1))
                for i, (q, pa, sa, pb, sb) in enumerate(two):
                    v.tensor_tensor(Gv[:, :, q], utmpA[:, i, :CJ],
                                    utmpB[:, i, :CJ],
                                    bor).then_inc(s_upk, 1)

            wglob = 0
            for pi, (d, k) in enumerate(PASSES):
                for w in range(NWP):
                    if w % WPC == 0:
                        ch = w // WPC
                        gch = pi * NCHUNK + ch
                        c0, ncc = chunk_cols(ch)
                        unpack_chunk(gch, gch % NIB, ncc)
                    if wglob >= NSB:
                        v.wait_ge(s_pe, wglob - NSB + 1)
                    col = d * NWP + w
                    v.tensor_scalar(
                        segt[wglob % NSB][:], ic_sb[:],
                        slf[:, col:col+1],
                        scf[:, col:col+1],
                        mybir.AluOpType.is_equal, mybir.AluOpType.mult,
                    ).then_inc(s_seg, 1)
                    wglob += 1
            nl = 0
            for d in range(2):
                for c in range(NDC):
                    ncols = min(DCH, SH - c * DCH)
                    for h in range(2):
                        ob = 2 * (nl % 2) + h
                        v.wait_ge(s_pmm, 2 * nl + h + 1)
                        if d == 1 and c == 0 and h == 0:
                            v.wait_ge(s_qt, 2)      # dir0 quant read obig
                        v.tensor_copy(
                            obig[:, h, c*DCH:c*DCH+ncols], pps[ob][:, :ncols],
                        ).then_inc(s_oact, 1)
                    nl += 1
                # dir tail: abs-max per out-channel, then int8 quantize
                v.wait_ge(s_oact, (d + 1) * 2 * NDC)
                for h in range(2):
                    v.tensor_reduce(amaxc[:, 2*d+h:2*d+h+1], obig[:, h, :],
                                    mybir.AxisListType.X, mybir.AluOpType.max,
                                    apply_absolute_value=True
                                    ).then_inc(s_qv, 1)
                v.wait_ge(s_qv, 4 * d + 2)
                v.tensor_scalar(rscl[:, 2*d:2*d+2], amaxc[:, 2*d:2*d+2],
                                1e-20, None, mybir.AluOpType.max
                                ).then_inc(s_qv, 1)
                v.wait_ge(s_qv, 4 * d + 3)
                v.reciprocal(rscl[:, 2*d:2*d+2], rscl[:, 2*d:2*d+2]
                             ).then_inc(s_qv, 1)
                v.wait_ge(s_qv, 4 * d + 4)
                if d == 1:
                    v.wait_ge(s_od, 48)             # dir0 out DMAs read oq
                for h in range(2):
                    v.tensor_scalar(oq[:, h*SH:(h+1)*SH], obig[:, h, :],
                                    rscl[:, 2*d+h:2*d+h+1], 127.0,
                                    mybir.AluOpType.mult, mybir.AluOpType.mult
                                    ).then_inc(s_qt, 1)

        # ============ SCALAR: page->stage copies + stage->hT DMAs
        @block.scalar
        def _(a):
            pglob = 0
            for pi, (d, k) in enumerate(PASSES):
                for p in range(NPAGE):
                    st = p // SPS
                    stglob = pi * NSTG + st
                    a.wait_ge(s_pe, wlast[pi, p] + 1)
                    if stglob >= NSTB and p % SPS == 0:
                        a.wait_ge(s_stg[stglob % NSTB], 16 * (stglob // NSTB))
                    a.copy(stage[stglob % NSTB][:, (p % SPS)*128:(p % SPS+1)*128],
                           pages[pglob % NPB]).then_inc(s_act, 1)
                    pglob += 1
                    if p % SPS == SPS - 1 or p == NPAGE - 1:
                        p0 = st * SPS
                        npg = p - p0 + 1
                        a.wait_ge(s_act, pglob)
                        a.dma_start(hT[d, k][:, p0*128:(p0+npg)*128],
                                    stage[stglob % NSTB][:, :npg*128]
                                    ).then_inc(s_stg[stglob % NSTB], 16)

        # ============ SYNC: sl/sc + input chunks, proj loads, out DMAs
        @block.sync
        def _(s):
            s.dma_start(slsb[:], sl_d[:]).then_inc(s_slsc, 16)
            s.dma_start(scsb[:], sc_d[:]).then_inc(s_slsc, 16)
            gch = 0
            for pi, (d, k) in enumerate(PASSES):
                for ch in range(NCHUNK):
                    if gch >= NIB:
                        s.wait_ge(s_pe, chk_wins[gch - NIB])
                    ib = gch % NIB
                    c0, ncc = chunk_cols(ch)
                    poff = pi * NCALL * 40
                    for t8 in range(8):
                        s.dma_start(
                            pbuf[t8*16:(t8+1)*16, ib, :ncc*40],
                            g_d[:, poff + c0*40: poff + (c0+ncc)*40],
                        ).then_inc(s_inb[ib], 16)
                    gch += 1
            # wait all stage->hT DMAs before projection loads
            NSTGALL = len(PASSES) * NSTG
            for b in range(NSTB):
                occ = (NSTGALL - b + NSTB - 1) // NSTB
                s.wait_ge(s_stg[b], 16 * occ)
            nl = 0
            for d in range(2):
                for c in range(NDC):
                    ncols = min(DCH, SH - c * DCH)
                    if nl >= 2:
                        s.wait_ge(s_pmm, 2 * (nl - 1))
                    for rk in range(R * NK):
                        r, k = rk // NK, rk % NK
                        s.dma_start(
                            prhs[nl % 2][:, rk, :ncols],
                            hT[d, k][:, r*PPR*128 + c*DCH: r*PPR*128 + c*DCH + ncols]
                        ).then_inc(s_pin[nl % 2], 16)
                    nl += 1
                s.wait_ge(s_qt, 2 * (d + 1))
                for h in range(2):
                    s.dma_start(outT_d[d, h*128:(h+1)*128, :],
                                oq[:, h*SH:(h+1)*SH]).then_inc(s_od, 16)
                s.dma_start(oscale_d[d], amaxc[:, 2*d:2*d+2]).then_inc(s_od, 16)
            s.wait_ge(s_od, 96)

    nc.compile()
    return nc


# ======================================================================
# Self-contained kernel entry point.
# ======================================================================
from concourse.bass_utils import run_bass_kernel_spmd as _run_spmd

_CACHE = {}


def kernel(**inputs):
    """GCMC layer on 8 trn2 NeuronCores. Returns (drug_out, dis_out) f32."""
    dirs_sorted = _sorted_dirs(inputs)
    c0 = Cfg(50000, 1024, 8, wpp=1)
    mx = max_page_count(c0.SH, c0.NC, c0.PPR, dirs_sorted)
    wpp = max(1, (mx + 127) // 128)
    cfg = Cfg(50000, 1024, 8, wpp=wpp)
    maps = build_inputs(cfg, inputs, dirs_sorted)
    if wpp not in _CACHE:
        _CACHE[wpp] = build_kernel(cfg)
    res = _run_spmd(_CACHE[wpp], maps, list(range(cfg.NC)))
    return assemble_output(cfg, res.results, inputs["fc_b"])


# revision 5
# speedup vs baseline: 1.0501x; 1.0501x over previous
"""GCMC message-passing kernel for trn2: builder + host preprocessing.

Per core = one dst-shard, both directions (0: drug->dis, 1: dis->drug).
  Phase W: device computes W[r] = att @ basis -> Wtab[R, IN, 128pad] bf16 HBM.
  Phase E (x6 passes = 2 dirs x 3 k-feats): per-edge event streams sorted by
    slot (r-major, dst-local), 128-event windows, WPP windows per 128-slot
    page, per-rating window count padded to RW (mult of 8) so each 1024-event
    gather call is single-rating. dma_gather pulls 256B W rows from wtab[r]
    (1024 events/call); DVE builds SegT[128ev,128slot] = is_equal(IC,sl)*sc
    (sc = cj*ci, host-folded); PE: msgs.T @ SegT accumulated into a PSUM page
    [MU, 128]. Pages -> SBUF stage (ACT) -> hT[d,k] = [MU, NSLOT] bf16 HBM.
  Phase P: outT[d] [256, SH] = sum_rk fcblk_rk.T @ hT-slices + bias.
Host assembles + transposes the two outputs.

Wire-size optimizations (axon transfer is the wall-clock bottleneck):
  - gather indices: feat only (10 bits), 8 values packed in 5 int16,
    shipped untiled [16, .]; device unpacks with DVE int ops and rebuilds
    the DGE 16-partition-wrapped replicated layout via 8 partition-group
    DMAs. Rating is static per call -> per-rating gather table slice.
  - sl/sc shipped once per direction (identical across the 3 k-passes);
    sl as uint8, sc as bf16; converted to f32 once on device.
  - bf16 everywhere off-chip except f32 PSUM accumulation: basis, fc
    weights, h, outputs.
"""
import numpy as np
import jax
jax.config.update("jax_compilation_cache_dir", "/tmp/jaxcache")
jax.config.update("jax_persistent_cache_min_entry_size_bytes", -1)
jax.config.update("jax_persistent_cache_min_compile_time_secs", 0)
import concourse.bass as bass
import concourse.bacc as bacc
import concourse.mybir as mybir

F32 = mybir.dt.float32
BF16 = mybir.dt.bfloat16
I16 = mybir.dt.int16
U8 = mybir.dt.uint8
NPBF16 = mybir.dt.np(mybir.dt.bfloat16)

R = 5
MU = 64
OUT = 256
NK = 3


class Cfg:
    def __init__(self, n_nodes, in_units, n_cores, wpp):
        self.N = n_nodes
        self.IN = in_units
        self.NC = n_cores
        self.SH = n_nodes // n_cores
        self.PPR = (self.SH + 127) // 128
        self.NPAGE = R * self.PPR
        self.NSLOT = self.NPAGE * 128
        self.WPP = wpp
        self.RW = ((self.PPR * wpp + 7) // 8) * 8   # windows per rating
        self.NWP = R * self.RW
        self.NCALL = self.NWP // 8
        self.CPC = 8                          # gather calls per input chunk
        self.WPC = self.CPC * 8               # windows per chunk
        self.NCHUNK = (self.NCALL + self.CPC - 1) // self.CPC
        self.DCH = 512
        self.NDC = (self.SH + self.DCH - 1) // self.DCH
        self.SPS = 16
        self.NSTG = (self.NPAGE + self.SPS - 1) // self.SPS
        self.WROUND = (in_units * MU) // 2048
        assert (in_units * MU) % 2048 == 0


def _sorted_dirs(inputs):
    """Per direction, per rating: dst-sorted (dst, src)."""
    gi = lambda n: np.asarray(inputs[n], np.int64)
    src, dst = gi("src"), gi("dst")
    out = []
    for dkey, skey in ((dst, src), (src, dst)):
        parts = []
        for r in range(R):
            order = np.argsort(dkey[r], kind="stable")
            parts.append((dkey[r][order], skey[r][order]))
        out.append(parts)
    return out


def max_page_count(sh, ncores, ppr, dirs_sorted):
    """Largest event count landing on one (core, rating, local-page) bucket."""
    mx = 0
    for parts in dirs_sorted:
        for r in range(R):
            dk = parts[r][0]
            core = dk // sh
            key = core * ppr + ((dk - core * sh) >> 7)
            cnt = np.bincount(key, minlength=ncores * ppr)
            mx = max(mx, int(cnt.max()))
    return mx


def build_inputs(cfg, inputs, dirs_sorted):
    f32 = np.float32
    gf = lambda n: np.asarray(inputs[n], f32)
    gi = lambda n: np.asarray(inputs[n], np.int64)
    drug_feat, dis_feat = gi("drug_feat"), gi("dis_feat")
    cj_drug, ci_drug = gf("cj_drug"), gf("ci_drug")
    cj_dis, ci_dis = gf("cj_dis"), gf("ci_dis")
    att, basis = gf("att"), gf("basis")
    fc_w, fc_b = gf("fc_w"), gf("fc_b")
    IN, SH, PPR, WPP, RW = cfg.IN, cfg.SH, cfg.PPR, cfg.WPP, cfg.RW
    NWP, NCALL, NPAGE, NC = cfg.NWP, cfg.NCALL, cfg.NPAGE, cfg.NC

    attT = att.T.astype(NPBF16)
    basisf = basis.reshape(4, IN * MU).astype(NPBF16)
    # fcrT[m, rk, o] = fc_w[r*NK*MU + k*MU + m, o]
    fcrT = fc_w.reshape(R * NK, MU, OUT).transpose(1, 0, 2).astype(NPBF16)
    IC = np.tile(np.arange(128, dtype=f32)[None, :], (128, 1)).copy()

    dirspec = [(drug_feat, cj_drug, ci_dis), (dis_feat, cj_dis, ci_drug)]
    maps = [
        {"attT": attT, "basisf": basisf, "fcrT": fcrT, "ic": IC,
         "g": np.zeros((16, 2 * NK * NCALL * 40), np.int16),
         "sl": np.zeros((128, 2 * NWP), np.uint8),
         "sc": np.zeros((128, 2 * NWP), np.uint8)}
        for _ in range(NC)
    ]
    for d, (feat, cj, ci) in enumerate(dirspec):
        parts = dirs_sorted[d]
        bounds = [np.searchsorted(parts[r][0], np.arange(NC + 1) * SH)
                  for r in range(R)]
        for c in range(NC):
            lo = c * SH
            dcat, scat, rcat = [], [], []
            for r in range(R):
                b0, b1 = bounds[r][c], bounds[r][c + 1]
                dcat.append(parts[r][0][b0:b1])
                scat.append(parts[r][1][b0:b1])
                rcat.append(np.full(b1 - b0, r, np.int64))
            dcat = np.concatenate(dcat)
            scat = np.concatenate(scat)
            rcat = np.concatenate(rcat)
            slot = rcat * (PPR * 128) + (dcat - lo)
            sc = cj[scat, 0] * ci[dcat, 0]
            pageg = slot >> 7                       # rcat*PPR + local page
            counts = np.bincount(pageg, minlength=NPAGE)
            assert counts.max() <= WPP * 128, (
                f"page overflow {counts.max()} > {WPP*128}; raise WPP")
            starts = np.concatenate(([0], np.cumsum(counts)[:-1]))
            pos = np.arange(slot.size) - np.repeat(starts, counts)
            ploc = pageg - rcat * PPR
            w = rcat * RW + ploc * WPP + (pos >> 7)
            flat = w * 128 + (pos & 127)
            SL = np.zeros(NWP * 128, np.uint8)
            SL[flat] = (slot & 127).astype(np.uint8)
            SC = np.zeros(NWP * 128, np.uint8)
            SC[flat] = np.rint(sc * 255.0).astype(np.uint8)
            m = maps[c]
            m["sl"][:, d * NWP:(d + 1) * NWP] = SL.reshape(NWP, 128).T
            m["sc"][:, d * NWP:(d + 1) * NWP] = SC.reshape(NWP, 128).T
            for k in range(NK):
                G = np.zeros(NWP * 128, np.uint16)
                G[flat] = feat[scat, k].astype(np.uint16)
                pi = d * NK + k
                wr = G.reshape(NCALL, 64, 16).transpose(2, 0, 1)
                v8 = wr.reshape(16, NCALL, 8, 8)
                vv = [v8[..., i] for i in range(8)]
                s5 = np.empty((16, NCALL, 8, 5), np.uint16)
                s5[..., 0] = vv[0] | (vv[1] << 10)
                s5[..., 1] = (vv[1] >> 6) | (vv[2] << 4) | (vv[3] << 14)
                s5[..., 2] = (vv[3] >> 2) | (vv[4] << 8)
                s5[..., 3] = (vv[4] >> 8) | (vv[5] << 2) | (vv[6] << 12)
                s5[..., 4] = (vv[6] >> 4) | (vv[7] << 6)
                m["g"][:, pi * NCALL * 40:(pi + 1) * NCALL * 40] = (
                    s5.reshape(16, NCALL * 40).view(np.int16))
    return maps


def assemble_output(cfg, results, fc_b):
    outs = []
    for d in range(2):
        parts = []
        for c in range(cfg.NC):
            i8 = results[c]["outT"][d].astype(np.float32)      # [OUT, SH]
            s = np.asarray(results[c]["oscale"][d], np.float32)  # [128, 2]
            scale = np.concatenate([s[:, 0], s[:, 1]]) / 127.0   # [OUT]
            parts.append((i8 * scale[:, None]).T)
        outs.append(np.concatenate(parts, 0) + np.asarray(fc_b, np.float32)[None, :])
    return outs[1], outs[0]


def build_kernel(cfg):
    nc = bacc.Bacc(None, target_bir_lowering=False, debug=True)
    IN, NCALL, NWP, RW = cfg.IN, cfg.NCALL, cfg.NWP, cfg.RW
    NPAGE, WPP, SH, PPR = cfg.NPAGE, cfg.WPP, cfg.SH, cfg.PPR
    CPC, WPC, NCHUNK = cfg.CPC, cfg.WPC, cfg.NCHUNK
    NSLOT, DCH, NDC, SPS, NSTG = cfg.NSLOT, cfg.DCH, cfg.NDC, cfg.SPS, cfg.NSTG
    WROUND = cfg.WROUND
    PASSES = [(d, k) for d in range(2) for k in range(NK)]

    attT_d = nc.declare_dram_parameter("attT", [4, R], BF16, isOutput=False)
    basisf_d = nc.declare_dram_parameter("basisf", [4, IN * MU], BF16, isOutput=False)
    fcr_d = nc.declare_dram_parameter("fcrT", [MU, R * NK, OUT], BF16, isOutput=False)
    ic_d = nc.declare_dram_parameter("ic", [128, 128], F32, isOutput=False)
    g_d = nc.declare_dram_parameter("g", [16, 2 * NK * NCALL * 40], I16, isOutput=False)
    sl_d = nc.declare_dram_parameter("sl", [128, 2 * NWP], U8, isOutput=False)
    sc_d = nc.declare_dram_parameter("sc", [128, 2 * NWP], U8, isOutput=False)
    outT_d = nc.declare_dram_parameter("outT", [2, OUT, SH], mybir.dt.int8, isOutput=True)
    oscale_d = nc.declare_dram_parameter("oscale", [2, 128, 2], F32, isOutput=True)

    # gather rows must be 256B multiples: pad W rows to 128 bf16 (64 real)
    wtab = nc.dram_tensor("wtab", [R, IN, 128], BF16)
    wtab_r = [wtab[r:r+1].rearrange("r f m -> (r f) m") for r in range(R)]
    hT = nc.dram_tensor("hT", [2, NK, MU, NSLOT], BF16)

    attT_sb = nc.alloc_sbuf_tensor("attT_sb", [4, R], BF16)
    bchunk = nc.alloc_sbuf_tensor("bchunk", [4, 2048], BF16)
    wstage = nc.alloc_sbuf_tensor("wstage", [R, 2048], BF16)
    ic_sb = nc.alloc_sbuf_tensor("ic_sb", [128, 128], F32)
    fcr_sb = nc.alloc_sbuf_tensor("fcr_sb", [MU, R * NK, OUT], BF16)
    NIB = 2
    gbuf = nc.alloc_sbuf_tensor("gbuf", [128, NIB, CPC * 64], I16)
    pbuf = nc.alloc_sbuf_tensor("pbuf", [128, NIB, CPC * 40], I16)
    utmpA = nc.alloc_sbuf_tensor("utmpA", [128, 4, CPC * 8], I16)
    utmpB = nc.alloc_sbuf_tensor("utmpB", [128, 4, CPC * 8], I16)
    slsb = nc.alloc_sbuf_tensor("slsb", [128, 2 * NWP], U8)
    scsb = nc.alloc_sbuf_tensor("scsb", [128, 2 * NWP], U8)
    slf = nc.alloc_sbuf_tensor("slf", [128, 2 * NWP], F32)
    scf = nc.alloc_sbuf_tensor("scf", [128, 2 * NWP], F32)
    NMB = 4
    msgs = [nc.alloc_sbuf_tensor(f"msgs{i}", [128, 8, 128], BF16) for i in range(NMB)]
    NSB = 4
    segt = [nc.alloc_sbuf_tensor(f"segt{i}", [128, 128], BF16) for i in range(NSB)]
    NSTB = 2
    stage = [nc.alloc_sbuf_tensor(f"stage{i}", [MU, SPS * 128], BF16) for i in range(NSTB)]
    prhs = [nc.alloc_sbuf_tensor(f"prhs{i}", [MU, R * NK, DCH], BF16) for i in range(2)]
    obig = nc.alloc_sbuf_tensor("obig", [128, 2, SH], BF16)
    oq = nc.alloc_sbuf_tensor("oq", [128, 2 * SH], mybir.dt.int8)
    amaxc = nc.alloc_sbuf_tensor("amaxc", [128, 4], F32)
    rscl = nc.alloc_sbuf_tensor("rscl", [128, 4], F32)

    NPB = 4
    psA = nc.alloc_psum_tensor("psA", [128, 2048], F32)
    pages = [psA[0:MU, i * 512:i * 512 + 128] for i in range(NPB)]
    psB = nc.alloc_psum_tensor("psB", [128, 2048], F32)
    wps = psB[0:R, :]
    pps = [psB[:, j * 512:(j + 1) * 512] for j in range(4)]  # j = 2*(l%2)+h

    def page_of_window(w):
        rr = w // RW
        return rr * PPR + min((w % RW) // WPP, PPR - 1)

    def rating_of_call(c):
        return min((c * 8) // RW, R - 1)

    wlast = {}
    for pi in range(len(PASSES)):
        for w in range(NWP):
            wlast[pi, page_of_window(w)] = pi * NWP + w

    def chunk_cols(ch):
        c0 = ch * CPC
        return c0, min(CPC, NCALL - c0)

    # global chunk -> (global calls through chunk, global windows through chunk)
    chk_calls, chk_wins = [], []
    tc = tw = 0
    for pi in range(len(PASSES)):
        for ch in range(NCHUNK):
            c0, ncc = chunk_cols(ch)
            tc += ncc
            tw += ncc * 8
            chk_calls.append(tc)
            chk_wins.append(tw)

    sems = {}

    with nc.Block() as block:
        for name, n in [("gth", NMB), ("inb", NIB), ("stg", NSTB),
                        ("pin", 2)]:
            for i in range(n):
                sems[name, i] = nc.alloc_semaphore(f"s_{name}{i}")
        for name in ["wdma", "wout", "wmm", "wcp", "seg", "pe", "act", "pmm",
                     "oact", "slsc", "upk", "utm", "qv", "qt", "od"]:
            sems[name] = nc.alloc_semaphore(f"s_{name}")
        s_gth = [sems["gth", i] for i in range(NMB)]
        s_inb = [sems["inb", i] for i in range(NIB)]
        s_stg = [sems["stg", i] for i in range(NSTB)]
        s_pin = [sems["pin", i] for i in range(2)]
        s_wdma, s_wmm, s_wcp = sems["wdma"], sems["wmm"], sems["wcp"]
        s_wout = sems["wout"]
        s_seg, s_pe, s_act = sems["seg"], sems["pe"], sems["act"]
        s_pmm, s_oact = sems["pmm"], sems["oact"]
        s_slsc, s_upk, s_utm = sems["slsc"], sems["upk"], sems["utm"]
        s_qv, s_qt, s_od = sems["qv"], sems["qt"], sems["od"]

        # ============ GPSIMD: const + W-build DMAs, then gathers
        @block.gpsimd
        def _(g):
            g.dma_start(attT_sb[:], attT_d[:]).then_inc(s_wdma, 16)
            g.dma_start(ic_sb[:], ic_d[:]).then_inc(s_wdma, 16)
            g.dma_start(fcr_sb[:], fcr_d[:]).then_inc(s_wdma, 16)
            for n in range(WROUND):
                g.wait_ge(s_wcp, n)  # round n-1 psum copied
                g.dma_start(bchunk[:], basisf_d[:, n*2048:(n+1)*2048]).then_inc(s_wdma, 16)
                g.wait_ge(s_wcp, n + 1)
                g.dma_start(wtab[:, n*32:(n+1)*32, 0:MU],
                            wstage[:].rearrange("r (f m) -> r f m", m=MU)
                            ).then_inc(s_wout, 16)
            g.wait_ge(s_wout, WROUND * 16)  # all wtab writes landed
            ncall = 0
            for pi, (d, k) in enumerate(PASSES):
                for c in range(NCALL):
                    gch = pi * NCHUNK + c // CPC
                    g.wait_ge(s_upk, 8 * (gch + 1))
                    b = ncall % NMB
                    if ncall >= NMB:
                        g.wait_ge(s_pe, 8 * (ncall - NMB + 1))
                    g.dma_gather(
                        msgs[b][:], wtab_r[rating_of_call(c)],
                        gbuf[:, gch % NIB, (c % CPC) * 64:(c % CPC + 1) * 64],
                        1024, 1024, 128,
                    ).then_inc(s_gth[b], 16)
                    ncall += 1

        # ============ TENSOR: W MMs, window MMs, projection MMs
        @block.tensor
        def _(t):
            for n in range(WROUND):
                t.wait_ge(s_wdma, 48 + 16 * (n + 1))
                if n >= 1:
                    t.wait_ge(s_wcp, n)
                for i in range(4):
                    ins = t.matmul(wps[:, i*512:(i+1)*512], attT_sb[:],
                                   bchunk[:, i*512:(i+1)*512],
                                   start=True, stop=True)
                ins.then_inc(s_wmm, 1)
            wglob = 0
            for pi, (d, k) in enumerate(PASSES):
                for w in range(NWP):
                    p = page_of_window(w)
                    pglob = pi * NPAGE + p
                    first = ((w % RW) % WPP == 0) and ((w % RW) // WPP == p % PPR)
                    if first and pglob >= NPB:
                        t.wait_ge(s_act, pglob - NPB + 1)
                    b = (wglob // 8) % NMB
                    t.wait_ge(s_gth[b], 16 * (wglob // 8 // NMB + 1))
                    t.wait_ge(s_seg, wglob + 1)
                    t.matmul(pages[pglob % NPB],
                             msgs[b][:, w % 8, 0:MU],
                             segt[wglob % NSB][:],
                             start=first, stop=(wglob == wlast[pi, p]),
                             ).then_inc(s_pe, 1)
                    wglob += 1
            nl = 0
            for d in range(2):
                for c in range(NDC):
                    ncols = min(DCH, SH - c * DCH)
                    t.wait_ge(s_pin[nl % 2], 240 * (nl // 2 + 1))
                    if nl >= 2:
                        t.wait_ge(s_oact, 2 * (nl - 1))
                    for h in range(2):
                        for rk in range(R * NK):
                            ins = t.matmul(pps[2*(nl % 2)+h][:, :ncols],
                                           fcr_sb[:, rk, h*128:(h+1)*128],
                                           prhs[nl % 2][:, rk, :ncols],
                                           start=(rk == 0), stop=(rk == R*NK-1))
                        ins.then_inc(s_pmm, 1)
                    nl += 1

        # ============ VECTOR: W psum->sbuf copies, unpack, SegT builds
        @block.vector
        def _(v):
            for n in range(WROUND):
                v.wait_ge(s_wmm, n + 1)
                if n >= 1:
                    v.wait_ge(s_wout, 16 * n)
                v.tensor_copy(wstage[:], wps[:]).then_inc(s_wcp, 1)
            v.wait_ge(s_slsc, 32)
            v.tensor_copy(slf[:], slsb[:]).then_inc(s_slsc, 1)
            v.tensor_scalar(scf[:], scsb[:], 1.0 / 255.0, None,
                            mybir.AluOpType.mult).then_inc(s_slsc, 1)
            v.wait_ge(s_slsc, 34)

            band = mybir.AluOpType.bitwise_and
            bor = mybir.AluOpType.bitwise_or
            lsr = mybir.AluOpType.logical_shift_right
            lsl = mybir.AluOpType.logical_shift_left

            def unpack_chunk(gch, ib, ncc):
                v.wait_ge(s_inb[ib], 128 * (gch // NIB + 1))
                if gch >= NIB:
                    v.wait_ge(s_pe, chk_wins[gch - NIB])
                if gch >= 1:
                    v.wait_ge(s_upk, 8 * gch)     # prior chunk writers done
                P = pbuf[:, ib, :ncc*40].rearrange("p (cj f) -> p cj f", f=5)
                Gv = gbuf[:, ib, :ncc*64].rearrange("p (cj q) -> p cj q", q=8)
                CJ = ncc * 8
                # single-source lanes
                v.tensor_scalar(Gv[:, :, 0], P[:, :, 0], 1023, None,
                                band).then_inc(s_upk, 1)
                v.tensor_scalar(Gv[:, :, 2], P[:, :, 1], 4, 1023,
                                lsr, band).then_inc(s_upk, 1)
                v.tensor_scalar(Gv[:, :, 5], P[:, :, 3], 2, 1023,
                                lsr, band).then_inc(s_upk, 1)
                v.tensor_scalar(Gv[:, :, 7], P[:, :, 4], 6, 1023,
                                lsr, band).then_inc(s_upk, 1)
                # two-source lanes: B = (Pb << sb) & 1023; G = (Pa >> sa) | B
                two = [(1, 0, 10, 1, 6), (3, 1, 14, 2, 2),
                       (4, 2, 8, 3, 8), (6, 3, 12, 4, 4)]
                for i, (q, pa, sa, pb, sb) in enumerate(two):
                    v.tensor_scalar(utmpB[:, i, :CJ], P[:, :, pb], sb, 1023,
                                    lsl, band).then_inc(s_utm, 1)
                # mask A to its valid width: arithmetic-vs-logical srl safe
                for i, (q, pa, sa, pb, sb) in enumerate(two):
                    v.tensor_scalar(utmpA[:, i, :CJ], P[:, :, pa], sa,
                                    (1 << (16 - sa)) - 1, lsr,
                                    band).then_inc(s_utm, 1)
                v.wait_ge(s_utm, 8 * (gch + 1))
                for i, (q, pa, sa, pb, sb) in enumerate(two):
                    v.tensor_tensor(Gv[:, :, q], utmpA[:, i, :CJ],
                                    utmpB[:, i, :CJ],
                                    bor).then_inc(s_upk, 1)

            wglob = 0
            for pi, (d, k) in enumerate(PASSES):
                for w in range(NWP):
                    if w % WPC == 0:
                        ch = w // WPC
                        gch = pi * NCHUNK + ch
                        c0, ncc = chunk_cols(ch)
                        unpack_chunk(gch, gch % NIB, ncc)
                    if wglob >= NSB:
                        v.wait_ge(s_pe, wglob - NSB + 1)
                    col = d * NWP + w
                    v.tensor_scalar(
                        segt[wglob % NSB][:], ic_sb[:],
                        slf[:, col:col+1],
                        scf[:, col:col+1],
                        mybir.AluOpType.is_equal, mybir.AluOpType.mult,
                    ).then_inc(s_seg, 1)
                    wglob += 1
            nl = 0
            for d in range(2):
                for c in range(NDC):
                    ncols = min(DCH, SH - c * DCH)
                    for h in range(2):
                        ob = 2 * (nl % 2) + h
                        v.wait_ge(s_pmm, 2 * nl + h + 1)
                        if d == 1 and c == 0 and h == 0:
                            v.wait_ge(s_qt, 2)      # dir0 quant read obig
                        v.tensor_copy(
                            obig[:, h, c*DCH:c*DCH+ncols], pps[ob][:, :ncols],
                        ).then_inc(s_oact, 1)
                    nl += 1
                # dir tail: abs-max per out-channel, then int8 quantize
                v.wait_ge(s_oact, (d + 1) * 2 * NDC)
                for h in range(2):
                    v.tensor_reduce(amaxc[:, 2*d+h:2*d+h+1], obig[:, h, :],
                                    mybir.AxisListType.X, mybir.AluOpType.max,
                                    apply_absolute_value=True
                                    ).then_inc(s_qv, 1)
                v.wait_ge(s_qv, 4 * d + 2)
                v.tensor_scalar(rscl[:, 2*d:2*d+2], amaxc[:, 2*d:2*d+2],
                                1e-20, None, mybir.AluOpType.max
                                ).then_inc(s_qv, 1)
                v.wait_ge(s_qv, 4 * d + 3)
                v.reciprocal(rscl[:, 2*d:2*d+2], rscl[:, 2*d:2*d+2]
                             ).then_inc(s_qv, 1)
                v.wait_ge(s_qv, 4 * d + 4)
                if d == 1:
                    v.wait_ge(s_od, 48)             # dir0 out DMAs read oq
                for h in range(2):
                    v.tensor_scalar(oq[:, h*SH:(h+1)*SH], obig[:, h, :],
                                    rscl[:, 2*d+h:2*d+h+1], 127.0,
                                    mybir.AluOpType.mult, mybir.AluOpType.mult
                                    ).then_inc(s_qt, 1)

        # ============ SCALAR: page->stage copies + stage->hT DMAs
        @block.scalar
        def _(a):
            pglob = 0
            for pi, (d, k) in enumerate(PASSES):
                for p in range(NPAGE):
                    st = p // SPS
                    stglob = pi * NSTG + st
                    a.wait_ge(s_pe, wlast[pi, p] + 1)
                    if stglob >= NSTB and p % SPS == 0:
                        a.wait_ge(s_stg[stglob % NSTB], 16 * (stglob // NSTB))
                    a.copy(stage[stglob % NSTB][:, (p % SPS)*128:(p % SPS+1)*128],
                           pages[pglob % NPB]).then_inc(s_act, 1)
                    pglob += 1
                    if p % SPS == SPS - 1 or p == NPAGE - 1:
                        p0 = st * SPS
                        npg = p - p0 + 1
                        a.wait_ge(s_act, pglob)
                        a.dma_start(hT[d, k][:, p0*128:(p0+npg)*128],
                                    stage[stglob % NSTB][:, :npg*128]
                                    ).then_inc(s_stg[stglob % NSTB], 16)

        # ============ SYNC: sl/sc + input chunks, proj loads, out DMAs
        @block.sync
        def _(s):
            s.dma_start(slsb[:], sl_d[:]).then_inc(s_slsc, 16)
            s.dma_start(scsb[:], sc_d[:]).then_inc(s_slsc, 16)
            gch = 0
            for pi, (d, k) in enumerate(PASSES):
                for ch in range(NCHUNK):
                    if gch >= NIB:
                        s.wait_ge(s_pe, chk_wins[gch - NIB])
                    ib = gch % NIB
                    c0, ncc = chunk_cols(ch)
                    poff = pi * NCALL * 40
                    for t8 in range(8):
                        s.dma_start(
                            pbuf[t8*16:(t8+1)*16, ib, :ncc*40],
                            g_d[:, poff + c0*40: poff + (c0+ncc)*40],
                        ).then_inc(s_inb[ib], 16)
                    gch += 1
            # wait all stage->hT DMAs before projection loads
            NSTGALL = len(PASSES) * NSTG
            for b in range(NSTB):
                occ = (NSTGALL - b + NSTB - 1) // NSTB
                s.wait_ge(s_stg[b], 16 * occ)
            nl = 0
            for d in range(2):
                for c in range(NDC):
                    ncols = min(DCH, SH - c * DCH)
                    if nl >= 2:
                        s.wait_ge(s_pmm, 2 * (nl - 1))
                    for rk in range(R * NK):
                        r, k = rk // NK, rk % NK
                        s.dma_start(
                            prhs[nl % 2][:, rk, :ncols],
                            hT[d, k][:, r*PPR*128 + c*DCH: r*PPR*128 + c*DCH + ncols]
                        ).then_inc(s_pin[nl % 2], 16)
                    nl += 1
                s.wait_ge(s_qt, 2 * (d + 1))
                for h in range(2):
                    s.dma_start(outT_d[d, h*128:(h+1)*128, :],
                                oq[:, h*SH:(h+1)*SH]).then_inc(s_od, 16)
                s.dma_start(oscale_d[d], amaxc[:, 2*d:2*d+2]).then_inc(s_od, 16)
            s.wait_ge(s_od, 96)

    nc.compile()
    return nc


# ======================================================================
# Self-contained kernel entry point.
# ======================================================================
from concourse.bass_utils import run_bass_kernel_spmd as _run_spmd

_CACHE = {}


def kernel(**inputs):
    """GCMC layer on 8 trn2 NeuronCores. Returns (drug_out, dis_out) f32."""
    dirs_sorted = _sorted_dirs(inputs)
    c0 = Cfg(50000, 1024, 8, wpp=1)
    mx = max_page_count(c0.SH, c0.NC, c0.PPR, dirs_sorted)
    wpp = max(1, (mx + 127) // 128)
    cfg = Cfg(50000, 1024, 8, wpp=wpp)
    maps = build_inputs(cfg, inputs, dirs_sorted)
    if wpp not in _CACHE:
        _CACHE[wpp] = build_kernel(cfg)
    res = _run_spmd(_CACHE[wpp], maps, list(range(cfg.NC)))
    return assemble_output(cfg, res.results, inputs["fc_b"])
